# revision 4
# baseline (speedup 1.0000x reference)
"""Causal multi-head attention (B=8, S=1024, D=768, H=12, Dh=64) on 8 TRN2
NeuronCores, batch-parallel (one batch element per core).

Per-core Bass/Tile kernel:
  - x^T built on-chip via PE transposes.
  - Q^T/K^T in [d, s] layout (weight-pair stationary, x^T moving, f32r matmuls).
  - V in [t, d] layout (x^T stationary, Wv moving) stored bf16 with a fused
    ones-column per head so the PV matmul also produces softmax row sums.
  - Scores computed transposed S^T[t, s] = K·Q^T with causal skip per t-chunk;
    exp on ScalarE (scale=1/8 folded in; no max subtraction needed — scores
    are O(5) so exp cannot overflow); diagonal block masked by a 0/1 triangle.
  - ctx^T[65, s] accumulated over t-chunks; PE transpose + reciprocal +
    per-partition scalar mul normalizes and writes the output layout directly.
"""

import sys
from contextlib import ExitStack

for _p in ("/opt/trn_rl_repo", "/root/.axon_site/_ro/trn_rl_repo"):
    if _p not in sys.path:
        sys.path.append(_p)

import numpy as np

import concourse.bass as bass  # noqa: F401
import concourse.bacc as bacc
import concourse.mybir as mybir
import concourse.tile as tile
from concourse.bass import ts
from concourse.bass_utils import run_bass_kernel_spmd
from concourse.masks import make_identity, make_upper_triangular

FP32 = mybir.dt.float32
FP32R = mybir.dt.float32r
BF16 = mybir.dt.bfloat16

B, S, D, H, DH = 8, 1024, 768, 12, 64
P = 128
NS, NK = S // P, D // P  # 8 s-chunks, 6 k-tiles
NG = H // 2              # 6 head-pair groups for Q/K projections
VW = DH + 1              # 65: V columns + ones column
N_CORES = 8


def _build_tile_kernel(tc, outs, ins):
    nc = tc.nc
    x, Wq, Wk, Wv = ins["x"], ins["Wq"], ins["Wk"], ins["Wv"]
    out = outs["out"]

    x_t = x.rearrange("(ns p) d -> p ns d", p=P)
    out_t = out.rearrange("(ns p) d -> p ns d", p=P)

    ctx = ExitStack()
    with ctx:
        consts = ctx.enter_context(tc.tile_pool(name="consts", bufs=1))
        sb1 = ctx.enter_context(tc.tile_pool(name="sb1", bufs=1))
        xin = ctx.enter_context(tc.tile_pool(name="xin", bufs=3))
        ptp = ctx.enter_context(tc.tile_pool(name="ptp", bufs=3))
        ctxs = ctx.enter_context(tc.tile_pool(name="ctxs", bufs=2))
        recp = ctx.enter_context(tc.tile_pool(name="recp", bufs=4))
        ps_tr = ctx.enter_context(tc.tile_pool(name="ps_tr", bufs=2, space="PSUM"))
        ps_big = ctx.enter_context(tc.tile_pool(name="ps_big", bufs=2, space="PSUM"))
        ps_ctx = ctx.enter_context(tc.tile_pool(name="ps_ctx", bufs=1, space="PSUM"))

        ident = consts.tile([P, P], FP32)
        make_identity(nc, ident)
        maskT = consts.tile([P, P], BF16)
        make_upper_triangular(nc, maskT, val=1.0, diag=True)

        xT = sb1.tile([P, NK, S], BF16)
        Wq_sb = sb1.tile([P, NK, H * DH], BF16)
        Wk_sb = sb1.tile([P, NK, H * DH], BF16)
        Wv_sb = sb1.tile([P, NK, H * DH], BF16)
        QT = sb1.tile([P, NG, S], BF16)
        KT = sb1.tile([P, NG, S], BF16)
        Vp = sb1.tile([P, NS, H * VW], BF16)
        out_sb = sb1.tile([P, NS, D], FP32)

        win = ctx.enter_context(tc.tile_pool(name="win", bufs=3))
        for w_dram, w_sb in ((Wq, Wq_sb), (Wk, Wk_sb), (Wv, Wv_sb)):
            for h in range(H):
                wtmp = win.tile([P, NK, DH], FP32, tag="w")
                nc.sync.dma_start(
                    out=wtmp,
                    in_=w_dram[h].rearrange("(kt p) d -> p kt d", p=P),
                )
                # f32 -> bf16 cast on the otherwise idle GpSimd engine
                nc.gpsimd.tensor_copy(
                    out=w_sb[:, :, h * DH : (h + 1) * DH], in_=wtmp
                )

        nc.gpsimd.memset(
            Vp.rearrange("p ns (h w) -> p ns h w", w=VW)[:, :, :, DH:VW], 1.0
        )

        for ns in range(NS):
            xc = xin.tile([P, D], FP32, tag="xc")
            nc.sync.dma_start(out=xc, in_=x_t[:, ns, :])
            for kt in range(NK):
                ptile = ps_tr.tile([P, P], FP32, tag="tr")
                nc.tensor.transpose(ptile, xc[:, ts(kt, P)], ident)
                nc.vector.tensor_copy(xT[:, kt, ts(ns, P)], ptile)

        for w_sb, dstT in ((Wq_sb, QT), (Wk_sb, KT)):
            for g in range(NG):
                acc = ps_big.tile([P, S], FP32, tag="big")
                for kt in range(NK):
                    for c in range(2):
                        nc.tensor.matmul(
                            acc[:, ts(c, 512)],
                            w_sb[:, kt, ts(g, P)],
                            xT[:, kt, ts(c, 512)],
                            start=(kt == 0),
                            stop=(kt == NK - 1),
                        )
                nc.vector.tensor_copy(dstT[:, g, :], acc)

        for ns in range(NS):
            accv = ps_big.tile([P, D], FP32, tag="big")
            for kt in range(NK):
                for c0, cw in ((0, 512), (512, 256)):
                    nc.tensor.matmul(
                        accv[:, c0 : c0 + cw],
                        xT[:, kt, ts(ns, P)],
                        Wv_sb[:, kt, c0 : c0 + cw],
                        start=(kt == 0),
                        stop=(kt == NK - 1),
                    )
            nc.vector.tensor_copy(
                Vp.rearrange("p ns (h w) -> p ns h w", w=VW)[:, ns, :, 0:DH],
                accv.rearrange("p (h d) -> p h d", d=DH),
            )

        for h in range(H):
            po = (h % 2) * DH
            g = h // 2
            ctx_ps = ps_ctx.tile([VW, S], FP32, tag="ctx")
            for j in range(NS):
                s0 = j * P
                sext = S - s0
                sc = ps_big.tile([P, S], FP32, tag="big")
                for c in range((sext + 511) // 512):
                    cw = min(512, sext - c * 512)
                    nc.tensor.matmul(
                        sc[:, c * 512 : c * 512 + cw],
                        KT[po : po + DH, g, ts(j, P)],
                        QT[po : po + DH, g, s0 + c * 512 : s0 + c * 512 + cw],
                        start=True,
                        stop=True,
                    )
                ptile = ptp.tile([P, S], BF16, tag="pt")
                nc.scalar.activation(
                    out=ptile[:, 0:sext],
                    in_=sc[:, 0:sext],
                    func=mybir.ActivationFunctionType.Exp,
                    scale=0.125,
                )
                nc.vector.tensor_mul(ptile[:, 0:P], ptile[:, 0:P], maskT)
                bounds = sorted({b for b in (s0, 512, S) if s0 <= b <= S})
                for b0, b1 in zip(bounds[:-1], bounds[1:]):
                    nc.tensor.matmul(
                        ctx_ps[:, b0:b1],
                        Vp[:, j, h * VW : (h + 1) * VW],
                        ptile[:, b0 - s0 : b1 - s0],
                        start=(j == 0),
                        stop=(j == NS - 1),
                        skip_group_check=True,
                    )
            ctx_sb = ctxs.tile([VW, S], FP32, tag="ctxs")
            nc.vector.tensor_copy(ctx_sb, ctx_ps)
            for m in range(NS):
                trp = ps_tr.tile([P, VW], FP32, tag="tr")
                nc.tensor.transpose(trp, ctx_sb[:, ts(m, P)], ident[0:VW, 0:VW])
                rec = recp.tile([P, 1], FP32, tag="rec")
                nc.vector.reciprocal(rec, trp[:, DH:VW])
                nc.vector.tensor_scalar_mul(
                    out_sb[:, m, h * DH : (h + 1) * DH], trp[:, 0:DH], rec
                )

        for ns in range(NS):
            nc.sync.dma_start(out=out_t[:, ns, :], in_=out_sb[:, ns, :])


_NC = {}


def build_nc(reps=1):
    """Build + compile the per-core Bass program once per process.

    reps > 1 emits the body multiple times with all-engine barriers between
    repetitions — used only for marginal-time measurement in test harnesses.
    """
    if reps in _NC:
        return _NC[reps]
    nc = bacc.Bacc("TRN2", target_bir_lowering=False, debug=False)
    ins = {
        "x": nc.dram_tensor("x", [S, D], FP32, kind="ExternalInput").ap(),
        "Wq": nc.dram_tensor("Wq", [H, D, DH], FP32, kind="ExternalInput").ap(),
        "Wk": nc.dram_tensor("Wk", [H, D, DH], FP32, kind="ExternalInput").ap(),
        "Wv": nc.dram_tensor("Wv", [H, D, DH], FP32, kind="ExternalInput").ap(),
    }
    outs = {"out": nc.dram_tensor("out", [S, D], FP32, kind="ExternalOutput").ap()}
    with tile.TileContext(nc) as tc:
        for i in range(reps):
            if i:
                tc.strict_bb_all_engine_barrier()
            _build_tile_kernel(tc, outs, ins)
    nc.compile()
    _NC[reps] = nc
    return nc


def make_in_maps(x, Wq, Wk, Wv):
    x = np.ascontiguousarray(x, dtype=np.float32)
    Wq = np.ascontiguousarray(Wq, dtype=np.float32)
    Wk = np.ascontiguousarray(Wk, dtype=np.float32)
    Wv = np.ascontiguousarray(Wv, dtype=np.float32)
    return [
        {"x": np.ascontiguousarray(x[b]), "Wq": Wq, "Wk": Wk, "Wv": Wv}
        for b in range(B)
    ]


def kernel(x, Wq, Wk, Wv):
    nc = build_nc()
    res = run_bass_kernel_spmd(nc, make_in_maps(x, Wq, Wk, Wv), list(range(N_CORES)))
    return np.stack([res.results[b]["out"] for b in range(B)], axis=0)


# revision 23
# speedup vs baseline: 6.4838x; 6.4838x over previous
"""Causal multi-head attention (B=8, S=1024, D=768, H=12, Dh=64) on 8 TRN2
NeuronCores, batch-parallel (one batch element per core).

Per-core Bass/Tile kernel, structured for engine overlap:
  - x DMAs ride the SP HWDGE ring while W DMAs ride the ACT ring in parallel.
  - Per s-chunk: PE transposes x -> x^T (bf16), then immediately projects
    V chunks (x^T stationary, Wv moving) so PE starts ~2us into the kernel.
  - Per head-pair group g: Q^T/K^T projections (weight-pair stationary, x^T
    moving), then attention for the two heads — the ScalarE exp work of group
    g overlaps the PE projection work of group g+1.
  - Scores are computed transposed S^T[t, s] = K·Q^T with causal skip; exp on
    ScalarE (scale=1/8 folded in, no max subtraction — scores are O(5));
    diagonal block masked by a 0/1 triangle multiply.
  - V' carries a ones-column per head so the PV matmul accumulates softmax
    denominators in ctx^T row 64; a PE transpose + reciprocal + per-partition
    scalar mul normalizes straight into the output layout.
"""

import sys
from contextlib import ExitStack

for _p in ("/opt/trn_rl_repo", "/root/.axon_site/_ro/trn_rl_repo"):
    if _p not in sys.path:
        sys.path.append(_p)

import numpy as np

import concourse.bass as bass  # noqa: F401
import concourse.bacc as bacc
import concourse.mybir as mybir
import concourse.tile as tile
from concourse.bass import ts
from concourse.bass_utils import run_bass_kernel_spmd
from concourse.masks import make_identity, make_upper_triangular

FP32 = mybir.dt.float32
BF16 = mybir.dt.bfloat16

B, S, D, H, DH = 8, 1024, 768, 12, 64
P = 128
NS, NK = S // P, D // P  # 8 s-chunks, 6 k-tiles
NG = H // 2              # 6 head-pair groups
VW = DH + 1              # 65: V columns + ones column
N_CORES = 8


def _build_tile_kernel(tc, outs, ins):
    nc = tc.nc
    x, Wq, Wk, Wv = ins["x"], ins["Wq"], ins["Wk"], ins["Wv"]
    out = outs["out"]

    x_t = x.rearrange("(ns p) d -> p ns d", p=P)
    out_t = out.rearrange("(ns p) d -> p ns d", p=P)

    ctx = ExitStack()
    with ctx:
        consts = ctx.enter_context(tc.tile_pool(name="consts", bufs=1))
        sb1 = ctx.enter_context(tc.tile_pool(name="sb1", bufs=1))
        win = ctx.enter_context(tc.tile_pool(name="win", bufs=4))
        xin = ctx.enter_context(tc.tile_pool(name="xin", bufs=8))
        ptp = ctx.enter_context(tc.tile_pool(name="ptp", bufs=6))
        ctxs = ctx.enter_context(tc.tile_pool(name="ctxs", bufs=2))
        recp = ctx.enter_context(tc.tile_pool(name="recp", bufs=4))
        ps_tr = ctx.enter_context(tc.tile_pool(name="ps_tr", bufs=2, space="PSUM"))
        ps_sc = ctx.enter_context(tc.tile_pool(name="ps_sc", bufs=4, space="PSUM"))
        ps_ctx = ctx.enter_context(tc.tile_pool(name="ps_ctx", bufs=1, space="PSUM"))

        ident = consts.tile([P, P], FP32)
        make_identity(nc, ident)
        maskT = consts.tile([P, P], BF16)
        make_upper_triangular(nc, maskT, val=1.0, diag=True)

        xT = sb1.tile([P, NK, S], BF16)
        Wq_sb = sb1.tile([P, NK // 2, 2, H, DH], BF16)
        Wk_sb = sb1.tile([P, NK // 2, 2, H, DH], BF16)
        Wv_sb = sb1.tile([P, NK // 2, 2, H, DH], BF16)
        QT = sb1.tile([P, NG, S], BF16)
        KT = sb1.tile([P, NG, S], BF16)
        Vp = sb1.tile([P, NS, H * VW], BF16)
        out_sb = sb1.tile([P, NS, D], FP32)

        nc.gpsimd.memset(
            Vp.rearrange("p ns (h w) -> p ns h w", w=VW)[:, :, :, DH:VW], 1.0
        )

        def load_w_chunk(w_dram, w_sb, kt2, h0, h1):
            # Two consecutive D-rows per partition line: 512B-contiguous on
            # both DMA sides (full SDMA rate; <512B runs pay a 2x penalty).
            # Contraction K-tile (kt2, two) maps partition p to D-row
            # kt2*256 + 2p + two; x^T uses the same permuted order.
            nh = h1 - h0
            wtmp = win.tile([P, H // 2, 2 * DH], FP32, tag="w")
            # W DMAs ride the ACT HWDGE ring (x rides the SP ring)
            nc.scalar.dma_start(
                out=wtmp[:, 0:nh, :],
                in_=w_dram[h0:h1, kt2 * 256 : (kt2 + 1) * 256, :].rearrange(
                    "h (p two) d -> p h (two d)", two=2
                ),
            )
            # f32 -> bf16 cast, alternating Pool / DVE to halve the stream;
            # also reshuffles to [kt2, two, h, d] so matmul slices for a
            # K-tile (kt2, two) are contiguous (walrus: single free dim).
            eng = nc.gpsimd if (kt2 % 2 == 0) else nc.vector
            eng.tensor_copy(
                out=w_sb[:, kt2, :, h0:h1, :],
                in_=wtmp[:, 0:nh, :].rearrange("p h (two d) -> p two h d", two=2),
            )

        # Moderately sized W DMAs (per-DMA HWDGE overhead is ~0.6us),
        # first-half heads of all tensors first so group 0 unblocks early.
        # DMA emission order: first x chunks interleaved with first-half W
        # chunks (HWDGE descriptor generation is serialized at ~0.6us/DMA,
        # so order = availability order).
        xcs = []
        for ns in range(NS):
            xc = xin.tile([P, D], FP32, tag="xc")
            nc.gpsimd.dma_start(out=xc, in_=x_t[:, ns, :])
            xcs.append(xc)
            if ns < 3:
                for w_dram, w_sb in ((Wv, Wv_sb), (Wq, Wq_sb), (Wk, Wk_sb)):
                    load_w_chunk(w_dram, w_sb, ns, 0, 6)
        for w_dram, w_sb in ((Wv, Wv_sb), (Wq, Wq_sb), (Wk, Wk_sb)):
            for kt2 in range(3):
                load_w_chunk(w_dram, w_sb, kt2, 6, 12)

        # x transposes (permuted-D order to match the W layout)
        for ns in range(NS):
            xcv = xcs[ns].rearrange("p (kt2 q two) -> p kt2 two q", kt2=3, two=2)
            for kt in range(NK):
                kt2, two = divmod(kt, 2)
                ptile = ps_tr.tile([P, P], FP32, tag="tr", name="xtp")
                nc.tensor.transpose(ptile, xcv[:, kt2, two, :], ident)
                nc.vector.tensor_copy(xT[:, kt, ts(ns, P)], ptile)

        # ---- emission units for the software-pipelined main loop ----

        def vproj_unit(g, ns):
            def emit():
                accv = ps_sc.tile([P, 512], FP32, tag="sc")
                for kt in range(NK):
                    kt2, two = divmod(kt, 2)
                    nc.tensor.matmul(
                        accv[:, 0:P],
                        xT[:, kt, ts(ns, P)],
                        Wv_sb[:, kt2, two, 2 * g : 2 * g + 2, :],
                        start=(kt == 0),
                        stop=(kt == NK - 1),
                    )
                nc.vector.tensor_copy(
                    Vp.rearrange("p ns (h w) -> p ns h w", w=VW)[
                        :, ns, 2 * g : 2 * g + 2, 0:DH
                    ],
                    accv[:, 0:P].rearrange("p (h d) -> p h d", d=DH),
                )

            return emit

        def qkproj_unit(g, w_sb, dstT, c):
            def emit():
                acc = ps_sc.tile([P, 512], FP32, tag="sc")
                for kt in range(NK):
                    kt2, two = divmod(kt, 2)
                    nc.tensor.matmul(
                        acc[:, 0:512],
                        w_sb[:, kt2, two, 2 * g : 2 * g + 2, :],
                        xT[:, kt, ts(c, 512)],
                        start=(kt == 0),
                        stop=(kt == NK - 1),
                    )
                nc.vector.tensor_copy(dstT[:, g, ts(c, 512)], acc[:, 0:512])

            return emit

        def proj_units(g):
            units = [vproj_unit(g, ns) for ns in range(NS)]
            for w_sb, dstT in ((Wq_sb, QT), (Wk_sb, KT)):
                for c in range(2):
                    units.append(qkproj_unit(g, w_sb, dstT, c))
            return units

        def attention_units(h):
            po = (h % 2) * DH
            g = h // 2
            state = {}

            def score_unit(j):
                def emit():
                    if j == 0:
                        state["ctx"] = ps_ctx.tile([VW, S], FP32, tag="ctx", name="ctxps")
                    s0 = j * P
                    sext = S - s0
                    ptile = ptp.tile([P, S], BF16, tag="pt")
                    for c in range((sext + 511) // 512):
                        cw = min(512, sext - c * 512)
                        sc = ps_sc.tile([P, 512], FP32, tag="sc")
                        nc.tensor.matmul(
                            sc[:, 0:cw],
                            KT[po : po + DH, g, ts(j, P)],
                            QT[po : po + DH, g, s0 + c * 512 : s0 + c * 512 + cw],
                            start=True,
                            stop=True,
                        )
                        nc.scalar.activation(
                            out=ptile[:, c * 512 : c * 512 + cw],
                            in_=sc[:, 0:cw],
                            func=mybir.ActivationFunctionType.Exp,
                            scale=0.125,
                        )
                    # causal mask on the diagonal block
                    nc.vector.tensor_mul(ptile[:, 0:P], ptile[:, 0:P], maskT)
                    bounds = sorted({b for b in (s0, 512, S) if s0 <= b <= S})
                    for b0, b1 in zip(bounds[:-1], bounds[1:]):
                        nc.tensor.matmul(
                            state["ctx"][:, b0:b1],
                            Vp[:, j, h * VW : (h + 1) * VW],
                            ptile[:, b0 - s0 : b1 - s0],
                            start=(j == 0),
                            stop=(j == NS - 1),
                            skip_group_check=True,
                        )

                return emit

            def ctx_copy_unit():
                def emit():
                    ctx_sb = ctxs.tile([VW, S], FP32, tag="ctxs", name="ctxsb")
                    nc.vector.tensor_copy(ctx_sb, state["ctx"])
                    state["ctx_sb"] = ctx_sb

                return emit

            def norm_unit(m0):
                def emit():
                    for m in range(m0, m0 + 4):
                        trp = ps_tr.tile([P, P], FP32, tag="tr", name="trp")
                        nc.tensor.transpose(
                            trp[:, 0:VW],
                            state["ctx_sb"][:, ts(m, P)],
                            ident[0:VW, 0:VW],
                        )
                        rec = recp.tile([P, 1], FP32, tag="rec")
                        nc.vector.reciprocal(rec, trp[:, DH:VW])
                        nc.vector.tensor_scalar_mul(
                            out_sb[:, m, h * DH : (h + 1) * DH], trp[:, 0:DH], rec
                        )

                return emit

            units = [score_unit(j) for j in range(NS)]
            units.append(ctx_copy_unit())
            units += [norm_unit(0), norm_unit(4)]
            return units

        # Software pipeline: group g's projections emit interleaved with
        # group g-1's attention so ScalarE exp always overlaps PE matmuls.
        for gi in range(NG + 1):
            att = []
            if gi >= 1:
                att = attention_units(2 * (gi - 1)) + attention_units(2 * gi - 1)
            prj = proj_units(gi) if gi < NG else []
            # proportional round-robin merge
            na, np_ = len(att), len(prj)
            ia = ip = 0
            while ia < na or ip < np_:
                if ip * max(na, 1) <= ia * max(np_, 1):
                    if ip < np_:
                        prj[ip]()
                        ip += 1
                    else:
                        att[ia]()
                        ia += 1
                else:
                    if ia < na:
                        att[ia]()
                        ia += 1
                    else:
                        prj[ip]()
                        ip += 1

        for c0 in (0, 6 * DH):
            for ns in range(NS):
                nc.sync.dma_start(
                    out=out_t[:, ns, c0 : c0 + 6 * DH],
                    in_=out_sb[:, ns, c0 : c0 + 6 * DH],
                )


_NC = {}


def build_nc(reps=1):
    """Build + compile the per-core Bass program once per process.

    reps > 1 emits the body multiple times with all-engine barriers between
    repetitions — used only for marginal-time measurement in test harnesses.
    """
    if reps in _NC:
        return _NC[reps]
    nc = bacc.Bacc("TRN2", target_bir_lowering=False, debug=False)
    ins = {
        "x": nc.dram_tensor("x", [S, D], FP32, kind="ExternalInput").ap(),
        "Wq": nc.dram_tensor("Wq", [H, D, DH], FP32, kind="ExternalInput").ap(),
        "Wk": nc.dram_tensor("Wk", [H, D, DH], FP32, kind="ExternalInput").ap(),
        "Wv": nc.dram_tensor("Wv", [H, D, DH], FP32, kind="ExternalInput").ap(),
    }
    outs = {"out": nc.dram_tensor("out", [S, D], FP32, kind="ExternalOutput").ap()}
    with tile.TileContext(nc) as tc:
        for i in range(reps):
            if i:
                tc.strict_bb_all_engine_barrier()
            _build_tile_kernel(tc, outs, ins)
    nc.compile()
    _NC[reps] = nc
    return nc


def make_in_maps(x, Wq, Wk, Wv):
    x = np.ascontiguousarray(x, dtype=np.float32)
    Wq = np.ascontiguousarray(Wq, dtype=np.float32)
    Wk = np.ascontiguousarray(Wk, dtype=np.float32)
    Wv = np.ascontiguousarray(Wv, dtype=np.float32)
    return [
        {"x": np.ascontiguousarray(x[b]), "Wq": Wq, "Wk": Wk, "Wv": Wv}
        for b in range(B)
    ]


def kernel(x, Wq, Wk, Wv):
    nc = build_nc()
    res = run_bass_kernel_spmd(nc, make_in_maps(x, Wq, Wk, Wv), list(range(N_CORES)))
    return np.stack([res.results[b]["out"] for b in range(B)], axis=0)
